# revision 21
# baseline (speedup 1.0000x reference)
"""Trainium2 Bass kernel for BlittingStrokeModel (AA polyline rasterization).

Reference semantics: for each batch item, rasterize 16 AA line segments
(trajectory knots) onto a zero canvas via a point-to-segment distance field:
    dist = point-to-segment distance
    cov  = clip(line_width + 0.5 - dist, 0, 1)
    out  = max over segments, broadcast to 3 channels.

Device formulation (exact up to the reference's 1e-8/1e-12 epsilons). With
s = 1/sqrt(dd2), dd2 = dx^2+dy^2, dn2 = dd2/2:
    w   = (dx*x + dy*y - c0 - dn2) * s        # scaled, recentred dot product
    E   = relu(|w| - dn2*s)                   # segment-clamp excess / sqrt(dd2)
    Pp  = (dy*x - dx*y + cP) * s              # perpendicular line distance
    dist^2 = Pp^2 + E^2
    M   = min over segments of dist^2
    cov = clip(L + 0.5 - sqrt(M), 0, 1)
Max over segments of cov == cov(min dist) since cov is monotone in dist.

Engine split per (segment, 128-row stripe):
    ACT:   At = Abs(x*s0 + bias)              (plane + |.|)
    DVE:   E  = tensor_scalar(At, -dn2s, max 0)
    DVE:   d2 = custom fused op  sq(x*aP + bP) + sq(E)   -> directly min'd
    GPSIMD: M = min(M, d2)
The custom DVE op (P2SQ_ADD_SQ) is registered at runtime into
concourse.dve_ops.OPS, so this file is self-contained.

Sharding: data-parallel over batch, one image per NeuronCore (8 cores).
The output does not depend on the image *values*, so images never touch the
device; only tiny per-segment coefficient tables are uploaded.
"""

import numpy as np
from contextlib import ExitStack

B, C, H, W = 8, 3, 512, 512
K = 17
NSEG = K - 1
P = 128
NSTRIPE = H // P  # 4

# per-seg uniform scalar columns in `cs`
CS_DXS, CS_AP, CS_DN2S, CS_NDN2S = 0, 1, 2, 3
CS_PER_SEG = 4
CS_THR = CS_PER_SEG * NSEG
CS_COLS = CS_THR + 1

_state = {}


def _register_dve_op(name, spec, perf_en=False):
    import concourse.dve_ops as dve_ops
    from concourse.dve_ops import DveOp, OPS, _SUB_OPCODE_FOR_NAME, _CUSTOM_DVE_ROW_BASE
    from concourse.dve_spec import lower, _has_src1
    from concourse.dve_uop import DveOpSpec
    from concourse.dve_table_gen import dve_ver_for

    if name in _SUB_OPCODE_FOR_NAME:
        return next(o for o in OPS if o.name == name)
    row = _CUSTOM_DVE_ROW_BASE + len(OPS)
    assert row < 0x20
    _SUB_OPCODE_FOR_NAME[name] = row
    ver = dve_ver_for("TRN2")
    tmp = DveOpSpec(
        name=name, opcode=row, uops=lower(spec, ver=ver), rd1_en=_has_src1(spec)
    )
    op = DveOp(
        name, spec, subdim=False, uops_sha={ver: tmp.sha(ver)},
        perf_en={ver: True} if perf_en else {},
    )
    OPS.append(op)
    dve_ops.CUSTOM_DVE_SPECS[name] = spec
    return op


def _get_dve_ops():
    """Register (once) the two fused DVE ops:
      D2MIN: out = min((Idx*s0 + s1)^2 + Src0^2, Src1)
      D2:    out = (Idx*s0 + s1)^2 + Src0^2        (first segment, no min)
    Idx is the DVE free-dim index generator == the x coordinate, so the
    perpendicular-plane term costs no tensor input and Src1 can carry the
    running minimum."""
    if "ops" in _state:
        return _state["ops"]
    from concourse.dve_spec import Spec, Src0, Src1, C0, C1, sq, minn, Idx

    def _idx(in0):
        return np.arange(in0.shape[-1], dtype=np.float32)[None, :]

    d2min = _register_dve_op(
        "STROKE_D2MIN_ANT",
        Spec(
            body=minn(sq(Idx * C0 + C1) + sq(Src0), Src1),
            reference=lambda in0, in1, s0, s1, imm2: np.minimum(
                (_idx(in0) * s0 + s1) ** 2 + in0.astype(np.float32) ** 2, in1
            ).astype(np.float32),
        ),
    )
    d2first = _register_dve_op(
        "STROKE_D2_ANT",
        Spec(
            body=sq(Idx * C0 + C1) + sq(Src0),
            reference=lambda in0, in1, s0, s1, imm2: (
                (_idx(in0) * s0 + s1) ** 2 + in0.astype(np.float32) ** 2
            ).astype(np.float32),
        ),
    )
    from concourse.dve_spec import relu

    erelu = _register_dve_op(
        "STROKE_ERELU_ANT",
        Spec(
            body=relu(Src0 - C0),
            reference=lambda in0, in1, s0, s1, imm2: np.maximum(
                in0.astype(np.float32) - s0, 0.0
            ).astype(np.float32),
        ),
        perf_en=True,
    )
    _state["ops"] = (d2min, d2first, erelu)
    return _state["ops"]


def _build_program():
    import concourse.bass as bass
    import concourse.tile as tile
    from concourse import bacc, mybir

    dt = mybir.dt
    op = mybir.AluOpType
    af = mybir.ActivationFunctionType
    d2min_op, d2first_op, erelu_op = _get_dve_ops()

    nc = bacc.Bacc(
        "TRN2", target_bir_lowering=False, debug=False, num_devices=8
    )
    xt_d = nc.dram_tensor("xt", [P, W], dt.float32, kind="ExternalInput").ap()
    cs_d = nc.dram_tensor("cs", [P, CS_COLS], dt.float32, kind="ExternalInput").ap()
    cdw_d = nc.dram_tensor("cdw", [P, NSTRIPE * NSEG], dt.float32, kind="ExternalInput").ap()
    cbp_d = nc.dram_tensor("cbp", [P, NSTRIPE * NSEG], dt.float32, kind="ExternalInput").ap()
    out_d = nc.dram_tensor("out", [C, H, W], dt.float32, kind="ExternalOutput").ap()

    with tile.TileContext(nc) as tc, ExitStack() as ctx:
        const = ctx.enter_context(tc.tile_pool(name="const", bufs=1))
        xt = const.tile_from(xt_d)
        cs = const.tile_from(cs_d)
        cdw = const.tile_from(cdw_d)
        cbp = const.tile_from(cbp_d)

        work = ctx.enter_context(tc.tile_pool(name="work", bufs=4))
        mpool = ctx.enter_context(tc.tile_pool(name="m", bufs=3))
        opool = ctx.enter_context(tc.tile_pool(name="o", bufs=3))

        def seg_col(s, which):
            c = s * CS_PER_SEG + which
            return cs[:, c : c + 1]

        # warm the ACT function tables (Abs/Relu/Sqrt) on a tiny tile while
        # the constant DMAs are still in flight
        wu = opool.tile([P, 8], dt.float32, name="wu")
        nc.vector.memset(wu[:], 0.0)
        wu2 = opool.tile([P, 8], dt.float32, name="wu2")
        nc.scalar.activation(wu2[:], wu[:], af.Abs)
        nc.scalar.activation(wu2[:], wu[:], af.Relu)
        nc.scalar.activation(wu2[:], wu[:], af.Sqrt)

        for T in range(NSTRIPE):
            M = None
            for s in range(NSEG):
                col = T * NSEG + s
                # At = |(dx*x + dy*y - c0 - dn2) * s|   [scalar ACT]
                At = work.tile([P, W], dt.float32, tag="At")
                nc.scalar.activation(
                    At[:], xt[:], af.Abs,
                    bias=cdw[:, col : col + 1], scale=seg_col(s, CS_DXS),
                )
                # E = relu(At - dn2s); split between V-ts and ACT-Relu to
                # balance engine load (V: custom op is 2 uOp passes; ACT: At).
                E = work.tile([P, W], dt.float32, tag="E")
                if s % 8 < 3:
                    nc.scalar.activation(
                        E[:], At[:], af.Relu, bias=seg_col(s, CS_NDN2S)
                    )
                else:
                    nc.vector._custom_dve(
                        erelu_op, out=E[:], in0=At[:], s0=seg_col(s, CS_DN2S)
                    )
                # M' = min((aP*x + bP)^2 + E^2, M)   [one fused custom DVE op]
                Mn = mpool.tile([P, W], dt.float32, tag="M")
                if s == 0:
                    nc.vector._custom_dve(
                        d2first_op, out=Mn[:], in0=E[:],
                        s0=seg_col(s, CS_AP), s1=cbp[:, col : col + 1],
                    )
                else:
                    nc.vector._custom_dve(
                        d2min_op, out=Mn[:], in0=E[:], in1=M[:],
                        s0=seg_col(s, CS_AP), s1=cbp[:, col : col + 1],
                    )
                M = Mn

            # dist = sqrt(M); cov = clip(thr - dist, 0, 1)
            dist = opool.tile([P, W], dt.float32, tag="dist")
            nc.scalar.activation(dist[:], M[:], af.Sqrt)
            cov1 = opool.tile([P, W], dt.float32, tag="cov1")
            nc.vector.tensor_scalar(
                cov1[:], dist[:], -1.0, cs[:, CS_THR : CS_THR + 1],
                op0=op.mult, op1=op.add,
            )
            cov = opool.tile([P, W], dt.float32, tag="cov")
            nc.vector.tensor_scalar(
                cov[:], cov1[:], 0.0, 1.0, op0=op.max, op1=op.min
            )
            for c in range(C):
                nc.sync.dma_start(out_d[c, T * P : (T + 1) * P, :], cov[:])

    nc.compile()
    return nc


def _prep_inputs(trajectories, line_width):
    """Host-side per-segment coefficient tables (numpy, float64 -> float32)."""
    thr = float(np.asarray(line_width).item()) + 0.5
    xt = np.broadcast_to(
        np.arange(W, dtype=np.float64), (P, W)
    ).astype(np.float32)
    xy = np.asarray(trajectories, dtype=np.float64)[:, :, 1:3]  # [B, K, 2]
    yv = np.arange(H, dtype=np.float64).reshape(NSTRIPE, P)  # y = T*128 + p

    in_maps = []
    for b in range(xy.shape[0]):
        p0, p1 = xy[b, :-1], xy[b, 1:]
        d = p1 - p0
        # degenerate-segment guard (measure-zero with random inputs)
        degen = (d[:, 0] ** 2 + d[:, 1] ** 2) < 1e-12
        d[degen, 0] = 1e-6
        dx, dy = d[:, 0], d[:, 1]
        p0x, p0y = p0[:, 0], p0[:, 1]
        dd2 = dx * dx + dy * dy
        sq = 1.0 / np.sqrt(dd2)
        dn2 = dd2 / 2.0
        c0 = dx * p0x + dy * p0y
        cP = dx * p0y - dy * p0x

        # [T, p, s] -> [p, T*NSEG+s]
        cdw = (dy[None, None, :] * yv[:, :, None] - (c0 + dn2)[None, None, :]) * sq[None, None, :]
        cdw = np.transpose(cdw, (1, 0, 2)).reshape(P, NSTRIPE * NSEG)
        cbp = (-dx[None, None, :] * yv[:, :, None] + cP[None, None, :]) * sq[None, None, :]
        cbp = np.transpose(cbp, (1, 0, 2)).reshape(P, NSTRIPE * NSEG)

        cs = np.zeros((P, CS_COLS), dtype=np.float64)
        cs[:, CS_DXS : CS_PER_SEG * NSEG : CS_PER_SEG] = dx * sq
        cs[:, CS_AP : CS_PER_SEG * NSEG : CS_PER_SEG] = dy * sq
        cs[:, CS_DN2S : CS_PER_SEG * NSEG : CS_PER_SEG] = dn2 * sq
        cs[:, CS_NDN2S : CS_PER_SEG * NSEG : CS_PER_SEG] = -dn2 * sq
        cs[:, CS_THR] = thr

        in_maps.append(
            {
                "xt": xt,
                "cs": cs.astype(np.float32),
                "cdw": cdw.astype(np.float32),
                "cbp": cbp.astype(np.float32),
            }
        )
    return in_maps


def kernel(**inputs):
    from concourse.bass_utils import run_bass_kernel_spmd

    images = np.asarray(inputs["images"])
    trajectories = np.asarray(inputs["trajectories"])
    line_width = inputs["line_width"]
    assert images.shape == (B, C, H, W), images.shape

    if "nc" not in _state:
        _state["nc"] = _build_program()
    nc = _state["nc"]

    in_maps = _prep_inputs(trajectories, line_width)
    res = run_bass_kernel_spmd(nc, in_maps, list(range(B))).results
    out = np.stack([res[i]["out"] for i in range(B)], axis=0)
    return out.astype(np.float32)


if __name__ == "__main__":
    rng = np.random.default_rng(0)
    ins = {
        "images": rng.standard_normal((B, C, H, W)).astype(np.float32),
        "trajectories": np.concatenate(
            [
                np.broadcast_to(np.linspace(0, 1, K, dtype=np.float32), (B, K))[..., None],
                rng.uniform(0, W - 1, (B, K, 2)).astype(np.float32),
                np.ones((B, K, 1), np.float32),
            ],
            axis=-1,
        ),
        "line_width": 3,
    }
    out = kernel(**ins)
    print(out.shape, out.dtype, out.min(), out.max())


# revision 22
# speedup vs baseline: 1.0947x; 1.0947x over previous
"""Trainium2 Bass kernel for BlittingStrokeModel (AA polyline rasterization).

Reference semantics: for each batch item, rasterize 16 AA line segments
(trajectory knots) onto a zero canvas via a point-to-segment distance field:
    dist = point-to-segment distance
    cov  = clip(line_width + 0.5 - dist, 0, 1)
    out  = max over segments, broadcast to 3 channels.

Device formulation (exact up to the reference's 1e-8/1e-12 epsilons). With
s = 1/sqrt(dd2), dd2 = dx^2+dy^2, dn2 = dd2/2:
    w   = (dx*x + dy*y - c0 - dn2) * s        # scaled, recentred dot product
    E   = relu(|w| - dn2*s)                   # segment-clamp excess / sqrt(dd2)
    Pp  = (dy*x - dx*y + cP) * s              # perpendicular line distance
    dist^2 = Pp^2 + E^2
    M   = min over segments of dist^2
    cov = clip(L + 0.5 - sqrt(M), 0, 1)
Max over segments of cov == cov(min dist) since cov is monotone in dist.

Engine split per (segment, 128-row stripe):
    ACT:   At = Abs(x*s0 + bias)              (plane + |.|)
    DVE:   E  = tensor_scalar(At, -dn2s, max 0)
    DVE:   d2 = custom fused op  sq(x*aP + bP) + sq(E)   -> directly min'd
    GPSIMD: M = min(M, d2)
The custom DVE op (P2SQ_ADD_SQ) is registered at runtime into
concourse.dve_ops.OPS, so this file is self-contained.

Sharding: data-parallel over batch, one image per NeuronCore (8 cores).
The output does not depend on the image *values*, so images never touch the
device; only tiny per-segment coefficient tables are uploaded.
"""

import numpy as np
from contextlib import ExitStack

B, C, H, W = 8, 3, 512, 512
K = 17
NSEG = K - 1
P = 128
NSTRIPE = H // P  # 4

# per-seg uniform scalar columns in `cs`
CS_DXS, CS_AP, CS_DN2S, CS_NDN2S = 0, 1, 2, 3
CS_PER_SEG = 4
CS_THR = CS_PER_SEG * NSEG
CS_COLS = CS_THR + 1

_state = {}


def _register_dve_op(name, spec, perf_en=False):
    import concourse.dve_ops as dve_ops
    from concourse.dve_ops import DveOp, OPS, _SUB_OPCODE_FOR_NAME, _CUSTOM_DVE_ROW_BASE
    from concourse.dve_spec import lower, _has_src1
    from concourse.dve_uop import DveOpSpec
    from concourse.dve_table_gen import dve_ver_for

    if name in _SUB_OPCODE_FOR_NAME:
        return next(o for o in OPS if o.name == name)
    row = _CUSTOM_DVE_ROW_BASE + len(OPS)
    assert row < 0x20
    _SUB_OPCODE_FOR_NAME[name] = row
    ver = dve_ver_for("TRN2")
    tmp = DveOpSpec(
        name=name, opcode=row, uops=lower(spec, ver=ver), rd1_en=_has_src1(spec)
    )
    op = DveOp(
        name, spec, subdim=False, uops_sha={ver: tmp.sha(ver)},
        perf_en={ver: True} if perf_en else {},
    )
    OPS.append(op)
    dve_ops.CUSTOM_DVE_SPECS[name] = spec
    return op


def _get_dve_ops():
    """Register (once) the two fused DVE ops:
      D2MIN: out = min((Idx*s0 + s1)^2 + Src0^2, Src1)
      D2:    out = (Idx*s0 + s1)^2 + Src0^2        (first segment, no min)
    Idx is the DVE free-dim index generator == the x coordinate, so the
    perpendicular-plane term costs no tensor input and Src1 can carry the
    running minimum."""
    if "ops" in _state:
        return _state["ops"]
    from concourse.dve_spec import Spec, Src0, Src1, C0, C1, sq, minn, Idx

    def _idx(in0):
        return np.arange(in0.shape[-1], dtype=np.float32)[None, :]

    d2min = _register_dve_op(
        "STROKE_D2MIN_ANT",
        Spec(
            body=minn(sq(Idx * C0 + C1) + sq(Src0), Src1),
            reference=lambda in0, in1, s0, s1, imm2: np.minimum(
                (_idx(in0) * s0 + s1) ** 2 + in0.astype(np.float32) ** 2, in1
            ).astype(np.float32),
        ),
    )
    d2first = _register_dve_op(
        "STROKE_D2_ANT",
        Spec(
            body=sq(Idx * C0 + C1) + sq(Src0),
            reference=lambda in0, in1, s0, s1, imm2: (
                (_idx(in0) * s0 + s1) ** 2 + in0.astype(np.float32) ** 2
            ).astype(np.float32),
        ),
    )
    from concourse.dve_spec import relu

    erelu = _register_dve_op(
        "STROKE_ERELU_ANT",
        Spec(
            body=relu(Src0 - C0),
            reference=lambda in0, in1, s0, s1, imm2: np.maximum(
                in0.astype(np.float32) - s0, 0.0
            ).astype(np.float32),
        ),
        perf_en=True,
    )
    _state["ops"] = (d2min, d2first, erelu)
    return _state["ops"]


def _build_program():
    import concourse.bass as bass
    import concourse.tile as tile
    from concourse import bacc, mybir

    dt = mybir.dt
    op = mybir.AluOpType
    af = mybir.ActivationFunctionType
    d2min_op, d2first_op, erelu_op = _get_dve_ops()

    nc = bacc.Bacc(
        "TRN2", target_bir_lowering=False, debug=False, num_devices=8
    )
    xt_d = nc.dram_tensor("xt", [P, W], dt.float32, kind="ExternalInput").ap()
    cs_d = nc.dram_tensor("cs", [P, CS_COLS], dt.float32, kind="ExternalInput").ap()
    cdw_d = nc.dram_tensor("cdw", [P, NSTRIPE * NSEG], dt.float32, kind="ExternalInput").ap()
    cbp_d = nc.dram_tensor("cbp", [P, NSTRIPE * NSEG], dt.float32, kind="ExternalInput").ap()
    out_d = nc.dram_tensor("out", [C, H, W], dt.float32, kind="ExternalOutput").ap()

    with tile.TileContext(nc) as tc, ExitStack() as ctx:
        const = ctx.enter_context(tc.tile_pool(name="const", bufs=1))
        xt = const.tile_from(xt_d)
        cs = const.tile_from(cs_d)
        cdw = const.tile_from(cdw_d)
        cbp = const.tile_from(cbp_d)

        work = ctx.enter_context(tc.tile_pool(name="work", bufs=4))
        mpool = ctx.enter_context(tc.tile_pool(name="m", bufs=3))
        opool = ctx.enter_context(tc.tile_pool(name="o", bufs=3))

        def seg_col(s, which):
            c = s * CS_PER_SEG + which
            return cs[:, c : c + 1]

        # warm the ACT function tables (Abs/Relu/Sqrt) on a tiny tile while
        # the constant DMAs are still in flight
        wu = opool.tile([P, 8], dt.float32, name="wu")
        nc.vector.memset(wu[:], 0.0)
        wu2 = opool.tile([P, 8], dt.float32, name="wu2")
        nc.scalar.activation(wu2[:], wu[:], af.Abs)
        nc.scalar.activation(wu2[:], wu[:], af.Relu)
        nc.scalar.activation(wu2[:], wu[:], af.Sqrt)

        for T in range(NSTRIPE):
            M = None
            for s in range(NSEG):
                col = T * NSEG + s
                # At = |(dx*x + dy*y - c0 - dn2) * s|   [scalar ACT]
                At = work.tile([P, W], dt.float32, tag="At")
                nc.scalar.activation(
                    At[:], xt[:], af.Abs,
                    bias=cdw[:, col : col + 1], scale=seg_col(s, CS_DXS),
                )
                # E = relu(At - dn2s); split between V-ts and ACT-Relu to
                # balance engine load (V: custom op is 2 uOp passes; ACT: At).
                E = work.tile([P, W], dt.float32, tag="E")
                if s % 8 < 3:
                    nc.scalar.activation(
                        E[:], At[:], af.Relu, bias=seg_col(s, CS_NDN2S)
                    )
                else:
                    nc.vector.tensor_scalar(
                        E[:], At[:], seg_col(s, CS_DN2S), 0.0,
                        op0=op.subtract, op1=op.max,
                    )
                # M' = min((aP*x + bP)^2 + E^2, M)   [one fused custom DVE op]
                Mn = mpool.tile([P, W], dt.float32, tag="M")
                if s == 0:
                    nc.vector._custom_dve(
                        d2first_op, out=Mn[:], in0=E[:],
                        s0=seg_col(s, CS_AP), s1=cbp[:, col : col + 1],
                    )
                else:
                    nc.vector._custom_dve(
                        d2min_op, out=Mn[:], in0=E[:], in1=M[:],
                        s0=seg_col(s, CS_AP), s1=cbp[:, col : col + 1],
                    )
                M = Mn

            # dist = sqrt(M); cov = clip(thr - dist, 0, 1)
            dist = opool.tile([P, W], dt.float32, tag="dist")
            nc.scalar.activation(dist[:], M[:], af.Sqrt)
            cov1 = opool.tile([P, W], dt.float32, tag="cov1")
            nc.vector.tensor_scalar(
                cov1[:], dist[:], -1.0, cs[:, CS_THR : CS_THR + 1],
                op0=op.mult, op1=op.add,
            )
            cov = opool.tile([P, W], dt.float32, tag="cov")
            nc.vector.tensor_scalar(
                cov[:], cov1[:], 0.0, 1.0, op0=op.max, op1=op.min
            )
            for c in range(C):
                nc.sync.dma_start(out_d[c, T * P : (T + 1) * P, :], cov[:])

    nc.compile()
    return nc


def _prep_inputs(trajectories, line_width):
    """Host-side per-segment coefficient tables (numpy, float64 -> float32)."""
    thr = float(np.asarray(line_width).item()) + 0.5
    xt = np.broadcast_to(
        np.arange(W, dtype=np.float64), (P, W)
    ).astype(np.float32)
    xy = np.asarray(trajectories, dtype=np.float64)[:, :, 1:3]  # [B, K, 2]
    yv = np.arange(H, dtype=np.float64).reshape(NSTRIPE, P)  # y = T*128 + p

    in_maps = []
    for b in range(xy.shape[0]):
        p0, p1 = xy[b, :-1], xy[b, 1:]
        d = p1 - p0
        # degenerate-segment guard (measure-zero with random inputs)
        degen = (d[:, 0] ** 2 + d[:, 1] ** 2) < 1e-12
        d[degen, 0] = 1e-6
        dx, dy = d[:, 0], d[:, 1]
        p0x, p0y = p0[:, 0], p0[:, 1]
        dd2 = dx * dx + dy * dy
        sq = 1.0 / np.sqrt(dd2)
        dn2 = dd2 / 2.0
        c0 = dx * p0x + dy * p0y
        cP = dx * p0y - dy * p0x

        # [T, p, s] -> [p, T*NSEG+s]
        cdw = (dy[None, None, :] * yv[:, :, None] - (c0 + dn2)[None, None, :]) * sq[None, None, :]
        cdw = np.transpose(cdw, (1, 0, 2)).reshape(P, NSTRIPE * NSEG)
        cbp = (-dx[None, None, :] * yv[:, :, None] + cP[None, None, :]) * sq[None, None, :]
        cbp = np.transpose(cbp, (1, 0, 2)).reshape(P, NSTRIPE * NSEG)

        cs = np.zeros((P, CS_COLS), dtype=np.float64)
        cs[:, CS_DXS : CS_PER_SEG * NSEG : CS_PER_SEG] = dx * sq
        cs[:, CS_AP : CS_PER_SEG * NSEG : CS_PER_SEG] = dy * sq
        cs[:, CS_DN2S : CS_PER_SEG * NSEG : CS_PER_SEG] = dn2 * sq
        cs[:, CS_NDN2S : CS_PER_SEG * NSEG : CS_PER_SEG] = -dn2 * sq
        cs[:, CS_THR] = thr

        in_maps.append(
            {
                "xt": xt,
                "cs": cs.astype(np.float32),
                "cdw": cdw.astype(np.float32),
                "cbp": cbp.astype(np.float32),
            }
        )
    return in_maps


def kernel(**inputs):
    from concourse.bass_utils import run_bass_kernel_spmd

    images = np.asarray(inputs["images"])
    trajectories = np.asarray(inputs["trajectories"])
    line_width = inputs["line_width"]
    assert images.shape == (B, C, H, W), images.shape

    if "nc" not in _state:
        _state["nc"] = _build_program()
    nc = _state["nc"]

    in_maps = _prep_inputs(trajectories, line_width)
    res = run_bass_kernel_spmd(nc, in_maps, list(range(B))).results
    out = np.stack([res[i]["out"] for i in range(B)], axis=0)
    return out.astype(np.float32)


if __name__ == "__main__":
    rng = np.random.default_rng(0)
    ins = {
        "images": rng.standard_normal((B, C, H, W)).astype(np.float32),
        "trajectories": np.concatenate(
            [
                np.broadcast_to(np.linspace(0, 1, K, dtype=np.float32), (B, K))[..., None],
                rng.uniform(0, W - 1, (B, K, 2)).astype(np.float32),
                np.ones((B, K, 1), np.float32),
            ],
            axis=-1,
        ),
        "line_width": 3,
    }
    out = kernel(**ins)
    print(out.shape, out.dtype, out.min(), out.max())


# revision 23
# speedup vs baseline: 1.2285x; 1.1222x over previous
"""Trainium2 Bass kernel for BlittingStrokeModel (AA polyline rasterization).

Reference semantics: for each batch item, rasterize 16 AA line segments
(trajectory knots) onto a zero canvas via a point-to-segment distance field:
    dist = point-to-segment distance
    cov  = clip(line_width + 0.5 - dist, 0, 1)
    out  = max over segments, broadcast to 3 channels.

Device formulation (exact up to the reference's 1e-8/1e-12 epsilons). With
s = 1/sqrt(dd2), dd2 = dx^2+dy^2, dn2 = dd2/2:
    w   = (dx*x + dy*y - c0 - dn2) * s        # scaled, recentred dot product
    E   = relu(|w| - dn2*s)                   # segment-clamp excess / sqrt(dd2)
    Pp  = (dy*x - dx*y + cP) * s              # perpendicular line distance
    dist^2 = Pp^2 + E^2
    M   = min over segments of dist^2
    cov = clip(L + 0.5 - sqrt(M), 0, 1)
Max over segments of cov == cov(min dist) since cov is monotone in dist.

Per (segment, 128-row stripe) the engine split is:
    ACT: At = Abs(x*dxs + cdw)     [plane + abs]
    V/ACT: E = relu(At - dn2s)     [assignment balances engine load]
    V:   M' = min((aP*x+bP)^2 + E^2, M)   [one fused custom DVE op; the
         x plane comes from the DVE Idx generator, so Src1 carries M]

Input-specialized program structure: host geometry (fp64, conservative
margins) decides per (core, segment, stripe) whether the segment can
influence the stripe at all (skip otherwise) and whether its endpoint-cap
term can matter there (drop the At/E ops and feed E=0 otherwise).  All 8
cores run one SPMD program whose per-stripe slot counts are the max over
cores; cores with fewer jobs pad with neutral coefficients (d2 = 1e12).
Programs are cached per structure; the custom DVE ops are registered at
runtime so this file is self-contained.

Sharding: data-parallel over batch, one image per NeuronCore (8 cores).
The output does not depend on the image *values*, so images never touch
the device; only tiny per-segment coefficient tables are uploaded.
"""

import numpy as np
from contextlib import ExitStack

B, C, H, W = 8, 3, 512, 512
K = 17
NSEG = K - 1
P = 128
NSTRIPE = H // P  # 4
MARG = 2.0  # conservative skip margin in pixels (fp32 error << 1e-2)

_state = {}


# --------------------------------------------------------------------------
# custom DVE ops
# --------------------------------------------------------------------------

def _register_dve_op(name, spec):
    import concourse.dve_ops as dve_ops
    from concourse.dve_ops import DveOp, OPS, _SUB_OPCODE_FOR_NAME, _CUSTOM_DVE_ROW_BASE
    from concourse.dve_spec import lower, _has_src1
    from concourse.dve_uop import DveOpSpec
    from concourse.dve_table_gen import dve_ver_for

    if name in _SUB_OPCODE_FOR_NAME:
        return next(o for o in OPS if o.name == name)
    row = _CUSTOM_DVE_ROW_BASE + len(OPS)
    assert row < 0x20
    _SUB_OPCODE_FOR_NAME[name] = row
    ver = dve_ver_for("TRN2")
    tmp = DveOpSpec(
        name=name, opcode=row, uops=lower(spec, ver=ver), rd1_en=_has_src1(spec)
    )
    op = DveOp(name, spec, subdim=False, uops_sha={ver: tmp.sha(ver)})
    OPS.append(op)
    dve_ops.CUSTOM_DVE_SPECS[name] = spec
    return op


def _get_dve_ops():
    if "ops" in _state:
        return _state["ops"]
    from concourse.dve_spec import Spec, Src0, Src1, C0, C1, sq, minn, Idx

    def _idx(in0):
        return np.arange(in0.shape[-1], dtype=np.float32)[None, :]

    d2min = _register_dve_op(
        "STROKE_D2MIN_ANT",
        Spec(
            body=minn(sq(Idx * C0 + C1) + sq(Src0), Src1),
            reference=lambda in0, in1, s0, s1, imm2: np.minimum(
                (_idx(in0) * s0 + s1) ** 2 + in0.astype(np.float32) ** 2, in1
            ).astype(np.float32),
        ),
    )
    d2first = _register_dve_op(
        "STROKE_D2_ANT",
        Spec(
            body=sq(Idx * C0 + C1) + sq(Src0),
            reference=lambda in0, in1, s0, s1, imm2: (
                (_idx(in0) * s0 + s1) ** 2 + in0.astype(np.float32) ** 2
            ).astype(np.float32),
        ),
    )
    _state["ops"] = (d2min, d2first)
    return _state["ops"]


# --------------------------------------------------------------------------
# host geometry: which (segment, stripe) pairs can matter, per core
# --------------------------------------------------------------------------

def _segments(xy):
    """Guarded segment endpoints/deltas (fp64). xy: [K, 2]."""
    p0, p1 = xy[:-1].copy(), xy[1:].copy()
    d = p1 - p0
    degen = (d[:, 0] ** 2 + d[:, 1] ** 2) < 1e-12
    d[degen, 0] = 1e-6
    p1 = p0 + d
    return p0, p1, d


def _seg_rect_dist(p0, p1, ylo, yhi):
    """Distance from segment (p0,p1) to rect [0, W-1] x [ylo, yhi]."""
    def pt_in_rect(p):
        return (0.0 <= p[0] <= W - 1) and (ylo <= p[1] <= yhi)

    if pt_in_rect(p0) or pt_in_rect(p1):
        return 0.0

    def ptseg(p, s0, s1):
        d = s1 - s0
        dd = float(d @ d)
        if dd < 1e-18:
            return float(np.hypot(*(p - s0)))
        t = min(1.0, max(0.0, float((p - s0) @ d) / dd))
        return float(np.hypot(*(p - s0 - t * d)))

    def ccw(A, B, C):
        return (C[1] - A[1]) * (B[0] - A[0]) > (B[1] - A[1]) * (C[0] - A[0])

    def inter(A, B, C, D):
        return ccw(A, C, D) != ccw(B, C, D) and ccw(A, B, C) != ccw(A, B, D)

    corners = [
        np.array([0.0, ylo]), np.array([W - 1.0, ylo]),
        np.array([W - 1.0, yhi]), np.array([0.0, yhi]),
    ]
    best = np.inf
    for i in range(4):
        b0, b1 = corners[i], corners[(i + 1) % 4]
        if inter(p0, p1, b0, b1):
            return 0.0
        best = min(
            best,
            ptseg(p0, b0, b1), ptseg(p1, b0, b1),
            ptseg(b0, p0, p1), ptseg(b1, p0, p1),
        )
    return best


def _plan(trajectories, line_width):
    """Decide kept jobs and cap-need per (core, stripe); build the SPMD
    union structure and per-core slot assignments."""
    thr = float(np.asarray(line_width).item()) + 0.5
    xy = np.asarray(trajectories, dtype=np.float64)[:, :, 1:3]
    nb = xy.shape[0]
    R = thr + MARG
    FAR = 1500.0

    # jobs[b][T] = list of (seg, needs_cap) — cap-needing first
    jobs = [[[] for _ in range(NSTRIPE)] for _ in range(nb)]
    for b in range(nb):
        p0a, p1a, da = _segments(xy[b])
        for T in range(NSTRIPE):
            ylo, yhi = T * P + 0.0, T * P + P - 1.0
            full, line = [], []
            for s in range(NSEG):
                p0, p1, d = p0a[s], p1a[s], da[s]
                if _seg_rect_dist(p0, p1, ylo, yhi) > R:
                    continue
                dirv = d / max(float(np.hypot(*d)), 1e-9)
                cap = (
                    _seg_rect_dist(p0, p0 - dirv * FAR, ylo, yhi) <= R
                    or _seg_rect_dist(p1, p1 + dirv * FAR, ylo, yhi) <= R
                )
                (full if cap else line).append((s, cap))
            jobs[b][T] = full + line
    nj = tuple(
        max(1, max(len(jobs[b][T]) for b in range(nb))) for T in range(NSTRIPE)
    )
    ncap = tuple(
        max(sum(1 for _, cap in jobs[b][T] if cap) for b in range(nb))
        for T in range(NSTRIPE)
    )
    # E-op engine split: balance V vs ACT load (costs in ns per op)
    nslot, ncaps = sum(nj), sum(ncap)
    x = int(round((800 * nslot + 500 * ncaps + 1200 - 800 * ncaps) / 1200.0))
    x = max(0, min(ncaps, x))
    eact = []
    seen = 0
    for T in range(NSTRIPE):
        for j in range(nj[T]):
            if j < ncap[T]:
                eact.append(seen < x)
                seen += 1
            else:
                eact.append(False)
    struct = (nj, ncap, tuple(eact))
    return struct, jobs, thr


# --------------------------------------------------------------------------
# program build (per structure, cached)
# --------------------------------------------------------------------------

def _build_program(struct):
    import concourse.tile as tile
    from concourse import bacc, mybir

    dt = mybir.dt
    op = mybir.AluOpType
    af = mybir.ActivationFunctionType
    d2min_op, d2first_op = _get_dve_ops()
    nj, ncap, eact = struct
    nslot = sum(nj)

    nc = bacc.Bacc("TRN2", target_bir_lowering=False, debug=False, num_devices=8)
    xt_d = nc.dram_tensor("xt", [P, W], dt.float32, kind="ExternalInput").ap()
    # per-slot scalars: [dxs, aP, dn2s, ndn2s] *nslot + [thr]
    cs_d = nc.dram_tensor("cs", [P, 4 * nslot + 1], dt.float32, kind="ExternalInput").ap()
    cdw_d = nc.dram_tensor("cdw", [P, nslot], dt.float32, kind="ExternalInput").ap()
    cbp_d = nc.dram_tensor("cbp", [P, nslot], dt.float32, kind="ExternalInput").ap()
    out_d = nc.dram_tensor("out", [C, H, W], dt.float32, kind="ExternalOutput").ap()

    with tile.TileContext(nc) as tc, ExitStack() as ctx:
        const = ctx.enter_context(tc.tile_pool(name="const", bufs=1))
        xt = const.tile_from(xt_d)
        cs = const.tile_from(cs_d)
        cdw = const.tile_from(cdw_d)
        cbp = const.tile_from(cbp_d)
        Z = const.tile([P, W], dt.float32, name="Z")
        nc.gpsimd.memset(Z[:], 0.0)

        work = ctx.enter_context(tc.tile_pool(name="work", bufs=4))
        mpool = ctx.enter_context(tc.tile_pool(name="m", bufs=3))
        opool = ctx.enter_context(tc.tile_pool(name="o", bufs=3))

        # warm the ACT function tables while const DMAs are in flight
        wu = opool.tile([P, 8], dt.float32, name="wu")
        nc.vector.memset(wu[:], 0.0)
        wu2 = opool.tile([P, 8], dt.float32, name="wu2")
        nc.scalar.activation(wu2[:], wu[:], af.Abs)
        nc.scalar.activation(wu2[:], wu[:], af.Relu)
        nc.scalar.activation(wu2[:], wu[:], af.Sqrt)

        g = 0
        for T in range(NSTRIPE):
            M = None
            for j in range(nj[T]):
                c4 = 4 * g
                if j < ncap[T]:
                    At = work.tile([P, W], dt.float32, tag="At")
                    nc.scalar.activation(
                        At[:], xt[:], af.Abs,
                        bias=cdw[:, g : g + 1], scale=cs[:, c4 : c4 + 1],
                    )
                    E = work.tile([P, W], dt.float32, tag="E")
                    if eact[g]:
                        nc.scalar.activation(
                            E[:], At[:], af.Relu, bias=cs[:, c4 + 3 : c4 + 4]
                        )
                    else:
                        nc.vector.tensor_scalar(
                            E[:], At[:], cs[:, c4 + 2 : c4 + 3], 0.0,
                            op0=op.subtract, op1=op.max,
                        )
                else:
                    E = Z
                Mn = mpool.tile([P, W], dt.float32, tag="M")
                if M is None:
                    nc.vector._custom_dve(
                        d2first_op, out=Mn[:], in0=E[:],
                        s0=cs[:, c4 + 1 : c4 + 2], s1=cbp[:, g : g + 1],
                    )
                else:
                    nc.vector._custom_dve(
                        d2min_op, out=Mn[:], in0=E[:], in1=M[:],
                        s0=cs[:, c4 + 1 : c4 + 2], s1=cbp[:, g : g + 1],
                    )
                M = Mn
                g += 1

            dist = opool.tile([P, W], dt.float32, tag="dist")
            nc.scalar.activation(dist[:], M[:], af.Sqrt)
            cov1 = opool.tile([P, W], dt.float32, tag="cov1")
            nc.vector.tensor_scalar(
                cov1[:], dist[:], -1.0, cs[:, 4 * nslot : 4 * nslot + 1],
                op0=op.mult, op1=op.add,
            )
            cov = opool.tile([P, W], dt.float32, tag="cov")
            nc.vector.tensor_scalar(
                cov[:], cov1[:], 0.0, 1.0, op0=op.max, op1=op.min
            )
            for c in range(C):
                nc.sync.dma_start(out_d[c, T * P : (T + 1) * P, :], cov[:])

    nc.compile()
    return nc


# --------------------------------------------------------------------------
# host coefficient tables
# --------------------------------------------------------------------------

def _prep_inputs(trajectories, struct, jobs, thr):
    nj, ncap, _ = struct
    nslot = sum(nj)
    xy = np.asarray(trajectories, dtype=np.float64)[:, :, 1:3]
    nb = xy.shape[0]
    xt = np.broadcast_to(np.arange(W, dtype=np.float64), (P, W)).astype(np.float32)
    yv = np.arange(H, dtype=np.float64).reshape(NSTRIPE, P)

    in_maps = []
    for b in range(nb):
        p0a, p1a, da = _segments(xy[b])
        dx, dy = da[:, 0], da[:, 1]
        p0x, p0y = p0a[:, 0], p0a[:, 1]
        dd2 = dx * dx + dy * dy
        sq = 1.0 / np.sqrt(dd2)
        dn2 = dd2 / 2.0
        c0 = dx * p0x + dy * p0y
        cP = dx * p0y - dy * p0x

        cs = np.zeros((P, 4 * nslot + 1))
        cdw = np.zeros((P, nslot))
        cbp = np.zeros((P, nslot))
        g = 0
        for T in range(NSTRIPE):
            myjobs = jobs[b][T]
            for j in range(nj[T]):
                c4 = 4 * g
                if j < len(myjobs):
                    s, _cap = myjobs[j]
                    cs[:, c4 + 0] = dx[s] * sq[s]
                    cs[:, c4 + 1] = dy[s] * sq[s]
                    # E = 0 for slots beyond this core's cap jobs (t-clamp
                    # excess provably irrelevant in this stripe)
                    ncap_mine = sum(1 for _s, cp in myjobs if cp)
                    if j < ncap_mine:
                        cs[:, c4 + 2] = dn2[s] * sq[s]
                        cs[:, c4 + 3] = -dn2[s] * sq[s]
                    else:
                        cs[:, c4 + 2] = 1e30
                        cs[:, c4 + 3] = -1e30
                    cdw[:, g] = (dy[s] * yv[T] - (c0[s] + dn2[s])) * sq[s]
                    cbp[:, g] = (-dx[s] * yv[T] + cP[s]) * sq[s]
                else:
                    # neutral padding: d2 = 1e12, E = 0
                    cs[:, c4 + 0] = 0.0
                    cs[:, c4 + 1] = 0.0
                    cs[:, c4 + 2] = 1e30
                    cs[:, c4 + 3] = -1e30
                    cdw[:, g] = 0.0
                    cbp[:, g] = 1e6
                g += 1
        cs[:, 4 * nslot] = thr

        in_maps.append(
            {
                "xt": xt,
                "cs": cs.astype(np.float32),
                "cdw": cdw.astype(np.float32),
                "cbp": cbp.astype(np.float32),
            }
        )
    return in_maps


def kernel(**inputs):
    from concourse.bass_utils import run_bass_kernel_spmd

    images = np.asarray(inputs["images"])
    trajectories = np.asarray(inputs["trajectories"])
    line_width = inputs["line_width"]
    assert images.shape == (B, C, H, W), images.shape

    struct, jobs, thr = _plan(trajectories, line_width)
    progs = _state.setdefault("progs", {})
    if struct not in progs:
        progs[struct] = _build_program(struct)
    nc = progs[struct]

    in_maps = _prep_inputs(trajectories, struct, jobs, thr)
    res = run_bass_kernel_spmd(nc, in_maps, list(range(B))).results
    out = np.stack([res[i]["out"] for i in range(B)], axis=0)
    return out.astype(np.float32)


if __name__ == "__main__":
    rng = np.random.default_rng(0)
    ins = {
        "images": rng.standard_normal((B, C, H, W)).astype(np.float32),
        "trajectories": np.concatenate(
            [
                np.broadcast_to(np.linspace(0, 1, K, dtype=np.float32), (B, K))[..., None],
                rng.uniform(0, W - 1, (B, K, 2)).astype(np.float32),
                np.ones((B, K, 1), np.float32),
            ],
            axis=-1,
        ),
        "line_width": 3,
    }
    out = kernel(**ins)
    print(out.shape, out.dtype, out.min(), out.max())
